# revision 1
# baseline (speedup 1.0000x reference)
"""ACAR head (grouped ROI attention) Trainium2 Bass kernel.

Strategy: data-parallel over ROI groups. roi_inds has NUM_CLIPS=8 groups and
there are 8 NeuronCores, so core c owns group c (padded to a common Npad).
Attention never crosses groups, so there is no inter-core communication; the
host shards inputs / gathers outputs.

Compute dtype: float32r (TF32-like rounded fp32) — full-rate on the PE at
free-dim >= 256, ~1.5e-4 relative rounding.
"""

import os
import sys
import types

sys.path.insert(0, "/opt/trn_rl_repo")

import numpy as np
import ml_dtypes


def _install_ntff_hook():
    """The image's antenv package lacks axon_hooks; inject it so trace=True
    can capture NTFF profiles. Harmless if anything is missing."""
    try:
        import antenv  # noqa: F401
        from trn_agent_boot.trn_boot import _ntff_profile_via_ctypes

        hook = _ntff_profile_via_ctypes("/opt/axon/libaxon_pjrt.so")
        if hook is None:
            return False
        mod = types.ModuleType("antenv.axon_hooks")
        mod.get_axon_ntff_profile_hook = lambda: hook
        mod.set_axon_ntff_profile_hook = lambda h: None
        sys.modules["antenv.axon_hooks"] = mod
        return True
    except Exception:
        return False


import concourse.bass as bass
import concourse.bacc as bacc
import concourse.tile as tile
from concourse import mybir
from concourse.bass_utils import run_bass_kernel_spmd
from concourse.masks import make_identity

F32 = mybir.dt.float32
F32R = mybir.dt.float32r
BF16 = mybir.dt.bfloat16

N_CORES = 8
N, C, T, H, W = 256, 512, 4, 8, 8
HW = H * W
CC = C // 128          # c chunks
NA = 64                # attention row pad (group size must be <= 64)
GN_EPS = 1e-5

LAST_EXEC_NS = None


def _build(npad: int):
    nbk = npad // 8
    nc = bacc.Bacc("TRN2", target_bir_lowering=False, debug=False,
                   num_devices=N_CORES)

    # ---- dram parameters (per-core shards) ----
    xp_d = nc.dram_tensor("xp", [CC, 128, npad // 8, T, 100, 8], BF16,
                          kind="ExternalInput").ap()
    xr_d = nc.dram_tensor("xr", [CC, 128, T, npad // 8, 512], F32,
                          kind="ExternalInput").ap()
    wq_d = nc.dram_tensor("wq", [CC, 128, 9, C], BF16, kind="ExternalInput").ap()
    wk_d = nc.dram_tensor("wk", [CC, 128, 9, C], BF16, kind="ExternalInput").ap()
    wv_d = nc.dram_tensor("wv", [CC, 128, 9, C], BF16, kind="ExternalInput").ap()
    wc_d = nc.dram_tensor("wc", [CC, 128, 9, C], BF16, kind="ExternalInput").ap()
    mask_d = nc.dram_tensor("mask", [NA], F32, kind="ExternalInput").ap()
    gamma_d = nc.dram_tensor("gamma", [C], F32, kind="ExternalInput").ap()
    beta_d = nc.dram_tensor("beta", [C], F32, kind="ExternalInput").ap()
    out_d = nc.dram_tensor("out", [CC, 128, T, npad // 8, 512], F32,
                           kind="ExternalOutput").ap()

    def bcast_ap(src, n_part, extra):
        return bass.AP(tensor=src.tensor, offset=src.offset,
                       ap=[[0, n_part]] + extra)

    with tile.TileContext(nc) as tc:
        with (
            tc.tile_pool(name="singles", bufs=1) as singles,
            tc.tile_pool(name="dram", bufs=1, space="DRAM") as dpool,
        ):
            ident = singles.tile([128, 128], F32)
            make_identity(nc, ident)
            ident_bf = singles.tile([128, 128], BF16)
            nc.vector.tensor_copy(out=ident_bf, in_=ident)
            mask_b = singles.tile([128, NA], F32)
            nc.gpsimd.dma_start(out=mask_b,
                                in_=bcast_ap(mask_d, 128, [[1, NA]]))
            zeros1 = singles.tile([128, 1], F32)
            nc.vector.memset(zeros1, 0.0)
            eps_t = singles.tile([64, 1], F32)
            nc.vector.memset(eps_t, GN_EPS)
            gam = singles.tile([128, CC], F32)
            bet = singles.tile([128, CC], F32)
            for cc in range(CC):
                nc.sync.dma_start(out=gam[:, cc:cc + 1],
                                  in_=gamma_d[cc * 128:(cc + 1) * 128])
                nc.sync.dma_start(out=bet[:, cc:cc + 1],
                                  in_=beta_d[cc * 128:(cc + 1) * 128])
            # per-(i, pair) bn stats: partitions 0:64 = even locs, 64:128 = odd
            stats = singles.tile([128, 128, 6], F32)

            vsp = dpool.tile([T, 32, 128, C], F32)
            mvd = dpool.tile([128, 2], F32)
            gnd = dpool.tile([64, 2], F32)

            # ============ Fused conv(q,k,v) + attention, per t ============
            with (
                tc.tile_pool(name="wA", bufs=1) as wpool,
                tc.tile_pool(name="xA", bufs=1) as xpool,
                tc.tile_pool(name="qkvB", bufs=1) as qkv,
                tc.tile_pool(name="sbB", bufs=2) as pB,
                tc.tile_pool(name="psAB", bufs=1, space="PSUM") as psum,
            ):
                for t in range(T):
                    # x tiles for this t (shared by all three convs)
                    xs = {}
                    for nb in range(nbk):
                        for cc in range(CC):
                            xt = xpool.tile([128, 100, 8], BF16,
                                            tag=f"x{nb}_{cc}",
                                            name=f"x{nb}_{cc}")
                            nc.sync.dma_start(out=xt, in_=xp_d[cc, :, nb, t])
                            xs[(nb, cc)] = xt
                    # qkv half tiles [c, 32hw, 64n], zero pad rows
                    qkv_sb = {}
                    for name, wd in (("q", wq_d), ("k", wk_d), ("v", wv_d)):
                        w_sb = []
                        for cc in range(CC):
                            wt = wpool.tile([128, 9, C], BF16,
                                            tag=f"w{cc}", name=f"w{cc}")
                            nc.sync.dma_start(out=wt, in_=wd[cc])
                            w_sb.append(wt)
                        for half in range(2):
                            for cc in range(CC):
                                tl = qkv.tile([128, 32, NA], BF16,
                                              tag=f"{name}{half}{cc}",
                                              name=f"{name}{half}{cc}")
                                if npad < NA:
                                    nc.vector.tensor_copy(
                                        out=tl[:, :, npad:NA],
                                        in_=bass.AP(
                                            tensor=zeros1.tensor,
                                            offset=zeros1.offset,
                                            ap=[zeros1.ap[0], [0, 32],
                                                [0, NA - npad]]),
                                    )
                                qkv_sb[(name, half, cc)] = tl
                        for nb in range(nbk):
                            for oc in range(4):
                                ps = psum.tile([128, 512], F32, tag="cps",
                                               bufs=2)
                                for cc in range(CC):
                                    for s in range(9):
                                        dh, dw = s // 3, s % 3
                                        xt = xs[(nb, cc)]
                                        rhs = bass.AP(
                                            tensor=xt.tensor,
                                            offset=xt.offset
                                            + (dh * 10 + dw) * 8,
                                            ap=[xt.ap[0], [80, 8], [8, 8],
                                                [1, 8]],
                                        )
                                        nc.tensor.matmul(
                                            ps,
                                            lhsT=w_sb[cc][:, s,
                                                          oc * 128:(oc + 1) * 128],
                                            rhs=rhs,
                                            start=(cc == 0 and s == 0),
                                            stop=(cc == CC - 1 and s == 8),
                                        )
                                # copy PSUM (h,w,n) into the half tiles
                                for half in range(2):
                                    tl = qkv_sb[(name, half, oc)]
                                    src_ap = bass.AP(
                                        tensor=ps.tensor,
                                        offset=ps.offset + half * 32 * 8,
                                        ap=[ps.ap[0], [8, 32], [1, 8]])
                                    dst_ap = bass.AP(
                                        tensor=tl.tensor,
                                        offset=tl.offset + nb * 8,
                                        ap=[tl.ap[0], [NA, 32], [1, 8]])
                                    if half == 0:
                                        nc.vector.tensor_copy(out=dst_ap,
                                                              in_=src_ap)
                                    else:
                                        nc.scalar.copy(out=dst_ap,
                                                       in_=src_ap)
                    # ---- attention for this t ----
                    for half in range(2):
                        q_sb = [qkv_sb[("q", half, cc)] for cc in range(CC)]
                        k_sb = [qkv_sb[("k", half, cc)] for cc in range(CC)]
                        v_sb = [qkv_sb[("v", half, cc)] for cc in range(CC)]
                        for quad in range(8):
                            h4 = quad * 4
                            s_ab = []
                            for sub in range(2):
                                sp = psum.tile([128, 256], F32, tag=f"s{sub}",
                                               bufs=1)
                                for cc in range(CC):
                                    nc.tensor.matmul(
                                        sp,
                                        lhsT=q_sb[cc][:, h4 + 2 * sub:
                                                      h4 + 2 * sub + 2, :],
                                        rhs=k_sb[cc][:, h4:h4 + 4, :],
                                        start=(cc == 0), stop=(cc == CC - 1),
                                    )
                                s_ab.append(sp)
                            for pp in range(2):
                                s_ps = s_ab[pp]
                                pair = t * 32 + half * 16 + quad * 2 + pp
                                e_sb = pB.tile([128, 128], BF16, tag="e")
                                nc.gpsimd.memset(e_sb[0:64, 64:128], 0.0)
                                nc.gpsimd.memset(e_sb[64:128, 0:64], 0.0)
                                nm = pB.tile([128, 1], F32, tag="nm")
                                dsum = pB.tile([128, 1], F32, tag="d")
                                rr = pB.tile([128, 1], F32, tag="r")
                                for l in range(2):
                                    rs = slice(64 * l, 64 * l + 64)
                                    cs = slice(128 * pp + 64 * l,
                                               128 * pp + 64 * l + 64)
                                    sm = pB.tile([128, 64], F32, tag="sm")
                                    nc.vector.tensor_tensor(
                                        out=sm[rs], in0=s_ps[rs, cs],
                                        in1=mask_b[rs],
                                        op=mybir.AluOpType.add)
                                    nc.vector.tensor_reduce(
                                        out=nm[rs], in_=sm[rs],
                                        axis=mybir.AxisListType.X,
                                        op=mybir.AluOpType.max, negate=True)
                                    nc.scalar.activation(
                                        out=e_sb[rs, rs], in_=sm[rs],
                                        func=mybir.ActivationFunctionType.Exp,
                                        bias=nm[rs], scale=1.0,
                                        accum_out=dsum[rs])
                                nc.vector.reciprocal(out=rr, in_=dsum)

                                et_ps = psum.tile([128, 128], BF16,
                                                  tag="et_ps", bufs=1)
                                nc.tensor.transpose(et_ps, e_sb, ident_bf)
                                et = pB.tile([128, 128], BF16, tag="et")
                                nc.vector.tensor_copy(out=et, in_=et_ps)

                                vpair = pB.tile([128, C], BF16, tag="vp")
                                for cc in range(CC):
                                    vt_ps = psum.tile([128, 128], BF16,
                                                      tag="vt_ps", bufs=2)
                                    nc.tensor.transpose(
                                        vt_ps,
                                        v_sb[cc][:, h4 + 2 * pp:
                                                 h4 + 2 * pp + 2, :],
                                        ident_bf)
                                    nc.vector.tensor_copy(
                                        out=vpair[:, cc * 128:(cc + 1) * 128],
                                        in_=vt_ps)

                                av_ps = psum.tile([128, C], F32, tag="av",
                                                  bufs=1)
                                nc.tensor.matmul(av_ps, lhsT=et, rhs=vpair,
                                                 start=True, stop=True)
                                vb = pB.tile([128, C], F32, tag="vb")
                                nc.scalar.activation(
                                    out=vb, in_=av_ps,
                                    func=mybir.ActivationFunctionType.Copy,
                                    scale=rr)
                                nc.vector.bn_stats(out=stats[:, pair, :],
                                                   in_=vb)
                                nc.sync.dma_start(
                                    out=vsp[t, half * 16 + quad * 2 + pp],
                                    in_=vb)

                # ---- GroupNorm stats finalize ----
                mv = pB.tile([128, 2], F32, tag="mv")
                nc.vector.bn_aggr(out=mv, in_=stats)
                nc.sync.dma_start(out=mvd, in_=mv)
                mva = pB.tile([64, 2], F32, tag="mva")
                mvb = pB.tile([64, 2], F32, tag="mvb")
                nc.sync.dma_start(out=mva, in_=mvd[0:64])
                nc.sync.dma_start(out=mvb, in_=mvd[64:128])
                mu = pB.tile([64, 1], F32, tag="mu")
                nc.vector.tensor_add(mu, mva[:, 0:1], mvb[:, 0:1])
                nc.vector.tensor_scalar_mul(mu, mu, 0.5)
                dm = pB.tile([64, 1], F32, tag="dm")
                nc.vector.tensor_sub(dm, mva[:, 0:1], mvb[:, 0:1])
                nc.vector.tensor_scalar_mul(dm, dm, 0.5)
                nc.vector.tensor_mul(dm, dm, dm)
                var = pB.tile([64, 1], F32, tag="var")
                nc.vector.tensor_add(var, mva[:, 1:2], mvb[:, 1:2])
                nc.vector.tensor_scalar_mul(var, var, 0.5)
                nc.vector.tensor_add(var, var, dm)
                rstd = pB.tile([64, 1], F32, tag="rstd")
                nc.scalar.activation(out=rstd, in_=var,
                                     func=mybir.ActivationFunctionType.Sqrt,
                                     bias=eps_t, scale=1.0)
                nc.vector.reciprocal(out=rstd, in_=rstd)
                murstd = pB.tile([64, 1], F32, tag="murstd")
                nc.vector.tensor_mul(murstd, mu, rstd)
                gpack = pB.tile([64, 2], F32, tag="gpack")
                nc.vector.tensor_copy(out=gpack[:, 0:1], in_=rstd)
                nc.vector.tensor_copy(out=gpack[:, 1:2], in_=murstd)
                nc.sync.dma_start(out=gnd, in_=gpack)

            # broadcast (rstd, mu*rstd) along partitions: [128, 64, 2]
            abn = singles.tile([128, 64, 2], F32)
            nc.gpsimd.dma_start(out=abn,
                                in_=bcast_ap(gnd, 128, [[2, 64], [1, 2]]))

            # ================= Phase C: GN apply + Wc conv + residual ====
            with (
                tc.tile_pool(name="wC", bufs=1) as wpool,
                tc.tile_pool(name="vtC", bufs=2) as vtpool,
                tc.tile_pool(name="padC", bufs=1) as padpool,
                tc.tile_pool(name="ioC", bufs=3) as iopool,
                tc.tile_pool(name="psC", bufs=1, space="PSUM") as psC,
            ):
                wc_sb = []
                for cc in range(CC):
                    wt = wpool.tile([128, 9, C], BF16, tag=f"wc{cc}")
                    nc.sync.dma_start(out=wt, in_=wc_d[cc])
                    wc_sb.append(wt)
                # persistent pre-zeroed padded tiles (double-buffered manually)
                vpads = []
                for par in range(2):
                    row = []
                    for cc in range(CC):
                        vp = padpool.tile([128, 100, 8], BF16,
                                          tag=f"vp{par}_{cc}")
                        nc.vector.tensor_copy(
                            out=vp,
                            in_=bass.AP(tensor=zeros1.tensor,
                                        offset=zeros1.offset,
                                        ap=[zeros1.ap[0], [0, 100], [0, 8]]))
                        row.append(vp)
                    vpads.append(row)
                it_c = 0
                for t in range(T):
                    vt_big = [vtpool.tile([128, 32, 2, NA], BF16,
                                          tag=f"vt{cc}", name=f"vt{cc}")
                              for cc in range(CC)]
                    for pr in range(32):
                        vb_r = iopool.tile([128, C], F32, tag="vbr",
                                           bufs=6)
                        nc.scalar.dma_start(out=vb_r, in_=vsp[t, pr])
                        for cc in range(CC):
                            tp = psC.tile([128, 128], F32, tag="tp", bufs=2)
                            nc.tensor.transpose(
                                tp, vb_r[:, cc * 128:(cc + 1) * 128], ident)
                            nc.vector.tensor_copy(out=vt_big[cc][:, pr, :, :],
                                                  in_=tp)
                    for cc in range(CC):
                        vt = vt_big[cc]
                        rstd_b = bass.AP(tensor=abn.tensor, offset=abn.offset,
                                         ap=[abn.ap[0], [0, 32], [0, 2],
                                             [2, NA]])
                        murstd_b = bass.AP(tensor=abn.tensor,
                                           offset=abn.offset + 1,
                                           ap=[abn.ap[0], [0, 32], [0, 2],
                                               [2, NA]])
                        nc.vector.tensor_tensor(out=vt, in0=vt, in1=rstd_b,
                                                op=mybir.AluOpType.mult)
                        nc.vector.tensor_tensor(out=vt, in0=vt, in1=murstd_b,
                                                op=mybir.AluOpType.subtract)
                        nc.vector.tensor_scalar(
                            out=vt, in0=vt, scalar1=gam[:, cc:cc + 1],
                            scalar2=bet[:, cc:cc + 1],
                            op0=mybir.AluOpType.mult,
                            op1=mybir.AluOpType.add)
                        nc.scalar.activation(
                            out=vt, in_=vt,
                            func=mybir.ActivationFunctionType.Relu)
                    for nb in range(nbk):
                        par = it_c % 2
                        it_c += 1
                        for cc in range(CC):
                            src = bass.AP(
                                tensor=vt_big[cc].tensor,
                                offset=vt_big[cc].offset + nb * 8,
                                ap=[vt_big[cc].ap[0], [512, 8], [64, 8],
                                    [1, 8]])
                            dst = bass.AP(
                                tensor=vpads[par][cc].tensor,
                                offset=vpads[par][cc].offset + 11 * 8,
                                ap=[vpads[par][cc].ap[0], [80, 8], [8, 8],
                                    [1, 8]])
                            nc.vector.tensor_copy(out=dst, in_=src)
                        for oc in range(4):
                            ps = psC.tile([128, 512], F32, tag="cps", bufs=4)
                            for cc in range(CC):
                                for s in range(9):
                                    dh, dw = s // 3, s % 3
                                    vp = vpads[par][cc]
                                    rhs = bass.AP(
                                        tensor=vp.tensor,
                                        offset=vp.offset + (dh * 10 + dw) * 8,
                                        ap=[vp.ap[0], [80, 8], [8, 8],
                                            [1, 8]])
                                    nc.tensor.matmul(
                                        ps,
                                        lhsT=wc_sb[cc][:, s,
                                                       oc * 128:(oc + 1) * 128],
                                        rhs=rhs,
                                        start=(cc == 0 and s == 0),
                                        stop=(cc == CC - 1 and s == 8))
                            xr = iopool.tile([128, 512], F32, tag="xr")
                            nc.sync.dma_start(out=xr,
                                              in_=xr_d[oc, :, t, nb])
                            ob = iopool.tile([128, 512], F32, tag="cob")
                            nc.vector.tensor_add(ob, ps, xr)
                            nc.sync.dma_start(out=out_d[oc, :, t, nb],
                                               in_=ob)

    nc.compile()
    return nc


_BUILD_CACHE = {}


def kernel(x, roi_inds, Wq, Wk, Wv, Wc, gn_gamma, gn_beta):
    global LAST_EXEC_NS
    x = np.ascontiguousarray(np.asarray(x, dtype=np.float32))
    roi_inds = np.asarray(roi_inds, dtype=np.int32)
    n, c = x.shape[0], x.shape[1]
    assert (n, c) == (N, C) and x.shape[2:] == (T, H, W)

    # group ROIs per clip; core g <- group g
    order = np.argsort(roi_inds, kind="stable")
    groups = [order[roi_inds[order] == g] for g in range(N_CORES)]
    sizes = [len(g) for g in groups]
    max_sz = max(sizes)
    assert max_sz <= NA, f"group size {max_sz} > {NA} unsupported"
    npad = ((max_sz + 7) // 8) * 8

    scale = 1.0 / np.sqrt(np.float32(C))

    def prep_w(Wt, sc=1.0):
        # [O, C, 1, 3, 3] -> [CC, 128, 9, O]
        w = (np.asarray(Wt, np.float32)[:, :, 0] * sc)  # [O, C, 3, 3]
        w = w.transpose(1, 2, 3, 0).reshape(CC, 128, 9, C)
        return np.ascontiguousarray(w).astype(ml_dtypes.bfloat16)

    w_arrs = {
        "wq": prep_w(Wq, scale), "wk": prep_w(Wk), "wv": prep_w(Wv),
        "wc": prep_w(Wc),
    }
    gamma = np.ascontiguousarray(np.asarray(gn_gamma, np.float32))
    beta = np.ascontiguousarray(np.asarray(gn_beta, np.float32))

    in_maps = []
    for g in range(N_CORES):
        idx = groups[g]
        xg = np.zeros((npad, C, T, H, W), np.float32)
        xg[:sizes[g]] = x[idx]
        # xp: [CC, 128, npad, T, 10, 10] zero-padded -> [CC,128,npad,T,100]
        xcm = xg.transpose(1, 0, 2, 3, 4).reshape(CC, 128, npad, T, H, W)
        xpp = np.zeros((CC, 128, npad, T, 10, 10), np.float32)
        xpp[..., 1:9, 1:9] = xcm
        # -> [CC, 128, NBK, T, 100, 8] with n innermost (contiguous for PE)
        xpp = xpp.reshape(CC, 128, npad // 8, 8, T, 100)
        xp = np.ascontiguousarray(
            xpp.transpose(0, 1, 2, 4, 5, 3)).astype(ml_dtypes.bfloat16)
        # xr: [CC, 128, T, NBK, (h,w,n8)] matching conv PSUM column order
        xr = np.ascontiguousarray(
            xcm.reshape(CC, 128, npad // 8, 8, T, HW)
            .transpose(0, 1, 4, 2, 5, 3)).reshape(
                CC, 128, T, npad // 8, 512)
        mask = np.zeros(NA, np.float32)
        mask[sizes[g]:] = -1e30
        m = {"xp": xp, "xr": xr, "mask": mask,
             "gamma": gamma, "beta": beta}
        m.update(w_arrs)
        in_maps.append(m)

    if npad not in _BUILD_CACHE:
        _BUILD_CACHE[npad] = _build(npad)
    nc = _BUILD_CACHE[npad]

    trace = bool(int(os.environ.get("ACAR_TRACE", "0")))
    if trace:
        _install_ntff_hook()
    res = run_bass_kernel_spmd(nc, in_maps, core_ids=list(range(N_CORES)),
                               trace=trace)
    LAST_EXEC_NS = res.exec_time_ns

    out = np.empty((N, C, T, H, W), np.float32)
    for g in range(N_CORES):
        og = res.results[g]["out"]  # [CC, 128, T, NBK, (hw, n8)]
        og = og.reshape(C, T, npad // 8, HW, 8)
        og = og.transpose(2, 4, 0, 1, 3).reshape(npad, C, T, H, W)
        out[groups[g]] = og[:sizes[g]]
    return out



# revision 8
# speedup vs baseline: 1.0234x; 1.0234x over previous
"""ACAR head (grouped ROI attention) Trainium2 Bass kernel.

Strategy: data-parallel over ROI groups (8 groups -> 8 cores, padded to
npad=40 rows). Attention never crosses groups => no inter-core comm.

v2 restructure vs baseline:
 - attention output computed transposed (avT = v^T @ p^T) so virt lives in
   [c, t, n, loc] layout; kills all phase-C PE transposes and the fp32 DRAM
   spill of virt (kept resident in SBUF as bf16).
 - GroupNorm stats via per-(c,n) loc-reductions + ones-matmul partition fold.
 - softmax packed 3 locations per PE op (npad=40 geometry, M=N=120).
 - residual taken from the bf16 padded x tiles (drops the fp32 xr input).
"""

import os
import sys
import types

sys.path.insert(0, "/opt/trn_rl_repo")

import numpy as np
import ml_dtypes


def _install_ntff_hook():
    try:
        import antenv  # noqa: F401
        from trn_agent_boot.trn_boot import _ntff_profile_via_ctypes

        hook = _ntff_profile_via_ctypes("/opt/axon/libaxon_pjrt.so")
        if hook is None:
            return False
        mod = types.ModuleType("antenv.axon_hooks")
        mod.get_axon_ntff_profile_hook = lambda: hook
        mod.set_axon_ntff_profile_hook = lambda h: None
        sys.modules["antenv.axon_hooks"] = mod
        return True
    except Exception:
        return False


import concourse.bass as bass
import concourse.bacc as bacc
import concourse.tile as tile
from concourse import mybir
from concourse.bass_utils import run_bass_kernel_spmd
from concourse.masks import make_identity

F32 = mybir.dt.float32
BF16 = mybir.dt.bfloat16

N_CORES = 8
N, C, T, H, W = 256, 512, 4, 8, 8
HW = H * W
CC = C // 128
NPAD = 40              # group row pad (multiple of 8, >= max group size)
NB = NPAD // 8
GN_EPS = 1e-5
NELEM = C * T * HW     # per-sample GN element count

# location groups: 20 groups of 3 + 2 groups of 2 (3*40=120 <= 128 partitions)
GROUPS = [(3 * g, 3) for g in range(20)] + [(60, 2), (62, 2)]

LAST_EXEC_NS = None
USE_DMA_T = bool(int(os.environ.get("ACAR_DMA_T", "1")))


def _build():
    nc = bacc.Bacc("TRN2", target_bir_lowering=False, debug=False,
                   num_devices=N_CORES)

    xp_d = nc.dram_tensor("xp", [CC, 128, NB, T, 100, 8], BF16,
                          kind="ExternalInput").ap()
    wq_d = nc.dram_tensor("wq", [CC, 128, 9, C], BF16, kind="ExternalInput").ap()
    wk_d = nc.dram_tensor("wk", [CC, 128, 9, C], BF16, kind="ExternalInput").ap()
    wv_d = nc.dram_tensor("wv", [CC, 128, 9, C], BF16, kind="ExternalInput").ap()
    wc_d = nc.dram_tensor("wc", [CC, 128, 9, C], BF16, kind="ExternalInput").ap()
    maskb_d = nc.dram_tensor("maskb", [120, 120], F32, kind="ExternalInput").ap()
    gamma_d = nc.dram_tensor("gamma", [C], F32, kind="ExternalInput").ap()
    beta_d = nc.dram_tensor("beta", [C], F32, kind="ExternalInput").ap()
    out_d = nc.dram_tensor("out", [CC, 128, T, NB, 512], F32,
                           kind="ExternalOutput").ap()

    def bcast_ap(src, n_part, extra):
        return bass.AP(tensor=src.tensor, offset=src.offset,
                       ap=[[0, n_part]] + extra)

    with tile.TileContext(nc) as tc:
        with (
            tc.tile_pool(name="singles", bufs=1) as singles,
            tc.tile_pool(name="dram", bufs=1, space="DRAM") as dpool,
        ):
            ident = singles.tile([128, 128], F32)
            make_identity(nc, ident)
            ident_bf = singles.tile([128, 128], BF16)
            nc.vector.tensor_copy(out=ident_bf, in_=ident)
            mask_b = singles.tile([120, 120], F32)
            nc.sync.dma_start(out=mask_b, in_=maskb_d)
            ones_t = singles.tile([128, 1], F32)
            nc.vector.memset(ones_t, 1.0)
            eps_t = singles.tile([40, 1], F32)
            nc.vector.memset(eps_t, GN_EPS)
            gam = singles.tile([128, CC], F32)
            bet = singles.tile([128, CC], F32)
            for cc in range(CC):
                nc.sync.dma_start(out=gam[:, cc:cc + 1],
                                  in_=gamma_d[cc * 128:(cc + 1) * 128])
                nc.sync.dma_start(out=bet[:, cc:cc + 1],
                                  in_=beta_d[cc * 128:(cc + 1) * 128])

            # persistent across phases: virt [c, t, n, loc] bf16 + GN partials
            virt = [singles.tile([128, T, NPAD, HW], BF16, name=f"virt{cc}")
                    for cc in range(CC)]
            gnsum = [singles.tile([128, T, NPAD], F32, name=f"gnsum{cc}")
                     for cc in range(CC)]
            gnsq = [singles.tile([128, T, NPAD], F32, name=f"gnsq{cc}")
                    for cc in range(CC)]
            gnd = dpool.tile([NPAD, 2], F32)

            # ============ Phase A: conv(q,k,v) + attention per t ============
            with (
                tc.tile_pool(name="wA", bufs=1) as wpool,
                tc.tile_pool(name="xA", bufs=8) as xpool,
                tc.tile_pool(name="qkvA", bufs=1) as qkv,
                tc.tile_pool(name="sbA", bufs=2) as pB,
                tc.tile_pool(name="sqA", bufs=1) as sqpool,
                tc.tile_pool(name="psA", bufs=1, space="PSUM") as psum,
            ):
                for t in range(T):
                    qkv_sb = {}
                    for name, wd in (("q", wq_d), ("k", wk_d), ("v", wv_d)):
                        w_sb = []
                        for cc in range(CC):
                            wt = wpool.tile([128, 9, C], BF16,
                                            tag=f"w{cc}", name=f"w{cc}")
                            nc.sync.dma_start(out=wt, in_=wd[cc])
                            w_sb.append(wt)
                        for cc in range(CC):
                            if name == "v":
                                # flat + 48-col pad so 128-wide transpose
                                # slices at l0*NPAD stay in bounds
                                vt_ = qkv.tile([128, HW * NPAD + 48], BF16,
                                               tag=f"{name}{cc}",
                                               name=f"{name}{cc}")
                                nc.vector.memset(vt_[:, HW * NPAD:], 0.0)
                                qkv_sb[(name, cc)] = vt_
                            else:
                                qkv_sb[(name, cc)] = qkv.tile(
                                    [128, HW, NPAD], BF16,
                                    tag=f"{name}{cc}", name=f"{name}{cc}")
                        copy_i = 0
                        for nb in range(NB):
                            xs = []
                            for cc in range(CC):
                                xt = xpool.tile([128, 100, 8], BF16, tag="x")
                                nc.sync.dma_start(out=xt, in_=xp_d[cc, :, nb, t])
                                xs.append(xt)
                            for oc in range(4):
                                ps = psum.tile([128, 512], F32, tag="cps",
                                               bufs=2)
                                for cc in range(CC):
                                    for s in range(9):
                                        dh, dw = s // 3, s % 3
                                        xt = xs[cc]
                                        rhs = bass.AP(
                                            tensor=xt.tensor,
                                            offset=xt.offset + (dh * 10 + dw) * 8,
                                            ap=[xt.ap[0], [80, 8], [8, 8],
                                                [1, 8]],
                                        )
                                        nc.tensor.matmul(
                                            ps,
                                            lhsT=w_sb[cc][:, s,
                                                          oc * 128:(oc + 1) * 128],
                                            rhs=rhs,
                                            start=(cc == 0 and s == 0),
                                            stop=(cc == CC - 1 and s == 8),
                                        )
                                # copy PSUM [oc, (hw, n8)] -> tile [oc, hw, n]
                                tl = qkv_sb[(name, oc)]
                                src = bass.AP(tensor=ps.tensor, offset=ps.offset,
                                              ap=[ps.ap[0], [8, HW], [1, 8]])
                                dst = bass.AP(tensor=tl.tensor,
                                              offset=tl.offset + nb * 8,
                                              ap=[tl.ap[0], [NPAD, HW], [1, 8]])
                                if copy_i % 2 == 0:
                                    nc.vector.tensor_copy(out=dst, in_=src)
                                else:
                                    nc.scalar.copy(out=dst, in_=src)
                                copy_i += 1

                    # ---- attention for this t ----
                    for gi, (l0, gl) in enumerate(GROUPS):
                        m = gl * NPAD
                        s_ps = psum.tile([120, 120], F32, tag="s", bufs=1)
                        for cc in range(CC):
                            nc.tensor.matmul(
                                s_ps[0:m, 0:m],
                                lhsT=qkv_sb[("q", cc)][:, l0:l0 + gl, :],
                                rhs=qkv_sb[("k", cc)][:, l0:l0 + gl, :],
                                start=(cc == 0), stop=(cc == CC - 1),
                            )
                        sm = pB.tile([120, 120], F32, tag="sm")
                        nc.vector.tensor_tensor(
                            out=sm[0:m, 0:m], in0=s_ps[0:m, 0:m],
                            in1=mask_b[0:m, 0:m], op=mybir.AluOpType.add)
                        nm = pB.tile([120, 1], F32, tag="nm")
                        nc.vector.tensor_reduce(
                            out=nm[0:m], in_=sm[0:m, 0:m],
                            axis=mybir.AxisListType.X,
                            op=mybir.AluOpType.max, negate=True)
                        dsum = pB.tile([120, 1], F32, tag="d")
                        e_sb = pB.tile([120, 120], BF16, tag="e")
                        nc.scalar.activation(
                            out=e_sb[0:m, 0:m], in_=sm[0:m, 0:m],
                            func=mybir.ActivationFunctionType.Exp,
                            bias=nm[0:m], scale=1.0, accum_out=dsum[0:m])
                        rr = pB.tile([120, 1], F32, tag="r")
                        nc.vector.reciprocal(out=rr[0:m], in_=dsum[0:m])
                        nc.vector.tensor_scalar_mul(e_sb[0:m, 0:m],
                                                    e_sb[0:m, 0:m], rr[0:m])
                        et_ps = psum.tile([120, 120], BF16, tag="et", bufs=1)
                        nc.tensor.transpose(et_ps[0:m, 0:m], e_sb[0:m, 0:m],
                                            ident_bf[0:m, 0:m])
                        et = pB.tile([120, 120], BF16, tag="ets")
                        nc.scalar.copy(out=et[0:m, 0:m], in_=et_ps[0:m, 0:m])

                        for cc in range(CC):
                            v3 = pB.tile([128, 128], BF16, tag=f"v3_{cc}")
                            vtile = qkv_sb[("v", cc)]
                            if USE_DMA_T:
                                vsl = bass.AP(
                                    tensor=vtile.tensor,
                                    offset=vtile.offset + l0 * NPAD,
                                    ap=[vtile.ap[0], [1, 128]])
                                nc.sync.dma_start_transpose(
                                    out=v3, in_=vsl)
                            else:
                                vsl = bass.AP(
                                    tensor=vtile.tensor,
                                    offset=vtile.offset + l0 * NPAD,
                                    ap=[vtile.ap[0], [1, m]])
                                v3p = psum.tile([128, 128], BF16,
                                                tag=f"v3p{cc}", bufs=2)
                                nc.tensor.transpose(v3p[0:m], vsl,
                                                    ident_bf)
                                nc.vector.tensor_copy(out=v3[0:m],
                                                      in_=v3p[0:m])
                            av_ps = psum.tile([128, 120], F32,
                                              tag="av", bufs=2)
                            nc.tensor.matmul(av_ps[:, 0:m], lhsT=v3[0:m],
                                             rhs=et[0:m, 0:m],
                                             start=True, stop=True)
                            # write virt[c, t, n, loc]: src (l, i) dims
                            src = bass.AP(tensor=av_ps.tensor,
                                          offset=av_ps.offset,
                                          ap=[av_ps.ap[0], [NPAD, gl],
                                              [1, NPAD]])
                            vt = virt[cc]
                            dst = bass.AP(
                                tensor=vt.tensor,
                                offset=vt.offset + t * NPAD * HW + l0,
                                ap=[vt.ap[0], [1, gl], [HW, NPAD]])
                            nc.scalar.copy(out=dst, in_=src)

                    # ---- GN partial sums for this t ----
                    for cc in range(CC):
                        vsl = bass.AP(
                            tensor=virt[cc].tensor,
                            offset=virt[cc].offset + t * NPAD * HW,
                            ap=[virt[cc].ap[0], [HW, NPAD], [1, HW]])
                        nc.vector.tensor_reduce(
                            out=gnsum[cc][:, t], in_=vsl,
                            axis=mybir.AxisListType.X, op=mybir.AluOpType.add)
                        sq = sqpool.tile([128, NPAD, HW], BF16, tag="sq")
                        eng = nc.vector if cc % 2 == 0 else nc.gpsimd
                        eng.tensor_tensor(out=sq, in0=vsl, in1=vsl,
                                          op=mybir.AluOpType.mult)
                        nc.vector.tensor_reduce(
                            out=gnsq[cc][:, t], in_=sq,
                            axis=mybir.AxisListType.X, op=mybir.AluOpType.add)

                # ---- GN finalize ----
                gn_ps_s = psum.tile([NPAD, 1], F32, tag="gns", bufs=1)
                gn_ps_q = psum.tile([NPAD, 1], F32, tag="gnq", bufs=1)
                k = 0
                for cc in range(CC):
                    for t in range(T):
                        nc.tensor.matmul(gn_ps_s, lhsT=gnsum[cc][:, t],
                                         rhs=ones_t, start=(k == 0),
                                         stop=(k == CC * T - 1))
                        k += 1
                k = 0
                for cc in range(CC):
                    for t in range(T):
                        nc.tensor.matmul(gn_ps_q, lhsT=gnsq[cc][:, t],
                                         rhs=ones_t, start=(k == 0),
                                         stop=(k == CC * T - 1))
                        k += 1
                inv = 1.0 / NELEM
                mu = pB.tile([NPAD, 1], F32, tag="mu")
                nc.vector.tensor_scalar_mul(mu, gn_ps_s, inv)
                e2 = pB.tile([NPAD, 1], F32, tag="e2")
                nc.vector.tensor_scalar_mul(e2, gn_ps_q, inv)
                mu2 = pB.tile([NPAD, 1], F32, tag="mu2")
                nc.vector.tensor_mul(mu2, mu, mu)
                var = pB.tile([NPAD, 1], F32, tag="var")
                nc.vector.tensor_sub(var, e2, mu2)
                rstd = pB.tile([NPAD, 1], F32, tag="rstd")
                nc.scalar.activation(out=rstd, in_=var,
                                     func=mybir.ActivationFunctionType.Sqrt,
                                     bias=eps_t, scale=1.0)
                nc.vector.reciprocal(out=rstd, in_=rstd)
                murstd = pB.tile([NPAD, 1], F32, tag="murstd")
                nc.vector.tensor_mul(murstd, mu, rstd)
                gpack = pB.tile([NPAD, 2], F32, tag="gpack")
                nc.vector.tensor_copy(out=gpack[:, 0:1], in_=rstd)
                nc.vector.tensor_copy(out=gpack[:, 1:2], in_=murstd)
                nc.sync.dma_start(out=gnd, in_=gpack)

            # broadcast (rstd, mu*rstd) to all partitions: [128, NPAD, 2]
            abn = singles.tile([128, NPAD, 2], F32)
            nc.gpsimd.dma_start(out=abn,
                                in_=bcast_ap(gnd, 128, [[2, NPAD], [1, 2]]))

            # ============ Phase C: GN apply + Wc conv + residual ============
            with (
                tc.tile_pool(name="wC", bufs=1) as wpool,
                tc.tile_pool(name="xC", bufs=8) as xcpool,
                tc.tile_pool(name="padC", bufs=1) as padpool,
                tc.tile_pool(name="ioC", bufs=3) as iopool,
                tc.tile_pool(name="psC", bufs=1, space="PSUM") as psC,
            ):
                # GN apply + relu, in place on virt (bf16)
                for cc in range(CC):
                    vt = virt[cc]
                    vap = bass.AP(tensor=vt.tensor, offset=vt.offset,
                                  ap=[vt.ap[0], [NPAD * HW, T], [HW, NPAD],
                                      [1, HW]])
                    rb = bass.AP(tensor=abn.tensor, offset=abn.offset,
                                 ap=[abn.ap[0], [0, T], [2, NPAD], [0, HW]])
                    mb = bass.AP(tensor=abn.tensor, offset=abn.offset + 1,
                                 ap=[abn.ap[0], [0, T], [2, NPAD], [0, HW]])
                    eng = nc.vector if cc % 2 == 0 else nc.gpsimd
                    eng.tensor_tensor(out=vap, in0=vap, in1=rb,
                                      op=mybir.AluOpType.mult)
                    eng.tensor_tensor(out=vap, in0=vap, in1=mb,
                                      op=mybir.AluOpType.subtract)
                    nc.scalar.activation(
                        out=vap, in_=vap,
                        func=mybir.ActivationFunctionType.Relu,
                        bias=bet[:, cc:cc + 1], scale=gam[:, cc:cc + 1])

                wc_sb = []
                for cc in range(CC):
                    wt = wpool.tile([128, 9, C], BF16, tag=f"wc{cc}")
                    nc.sync.dma_start(out=wt, in_=wc_d[cc])
                    wc_sb.append(wt)
                zeros1 = iopool.tile([128, 1], F32, tag="z", bufs=1)
                nc.vector.memset(zeros1, 0.0)
                vpads = []
                for par in range(2):
                    row = []
                    for cc in range(CC):
                        vp = padpool.tile([128, 100, 8], BF16,
                                          tag=f"vp{par}_{cc}")
                        nc.vector.tensor_copy(
                            out=vp,
                            in_=bass.AP(tensor=zeros1.tensor,
                                        offset=zeros1.offset,
                                        ap=[zeros1.ap[0], [0, 100], [0, 8]]))
                        row.append(vp)
                    vpads.append(row)
                it_c = 0
                for t in range(T):
                    for nb in range(NB):
                        par = it_c % 2
                        it_c += 1
                        xs = []
                        for cc in range(CC):
                            xt = xcpool.tile([128, 100, 8], BF16, tag="xc")
                            nc.sync.dma_start(out=xt, in_=xp_d[cc, :, nb, t])
                            xs.append(xt)
                        for cc in range(CC):
                            vt = virt[cc]
                            src = bass.AP(
                                tensor=vt.tensor,
                                offset=vt.offset + t * NPAD * HW + nb * 8 * HW,
                                ap=[vt.ap[0], [8, 8], [1, 8], [HW, 8]])
                            vp = vpads[par][cc]
                            dst = bass.AP(
                                tensor=vp.tensor,
                                offset=vp.offset + 11 * 8,
                                ap=[vp.ap[0], [80, 8], [8, 8], [1, 8]])
                            eng = nc.vector if cc % 2 == 0 else nc.gpsimd
                            eng.tensor_copy(out=dst, in_=src)
                        for oc in range(4):
                            ps = psC.tile([128, 512], F32, tag="cps", bufs=4)
                            for cc in range(CC):
                                for s in range(9):
                                    dh, dw = s // 3, s % 3
                                    vp = vpads[par][cc]
                                    rhs = bass.AP(
                                        tensor=vp.tensor,
                                        offset=vp.offset + (dh * 10 + dw) * 8,
                                        ap=[vp.ap[0], [80, 8], [8, 8],
                                            [1, 8]])
                                    nc.tensor.matmul(
                                        ps,
                                        lhsT=wc_sb[cc][:, s,
                                                       oc * 128:(oc + 1) * 128],
                                        rhs=rhs,
                                        start=(cc == 0 and s == 0),
                                        stop=(cc == CC - 1 and s == 8))
                            xsl = bass.AP(
                                tensor=xs[oc].tensor,
                                offset=xs[oc].offset + 11 * 8,
                                ap=[xs[oc].ap[0], [80, 8], [8, 8], [1, 8]])
                            psl = bass.AP(
                                tensor=ps.tensor, offset=ps.offset,
                                ap=[ps.ap[0], [64, 8], [8, 8], [1, 8]])
                            ob = iopool.tile([128, 512], F32, tag="cob")
                            obl = bass.AP(
                                tensor=ob.tensor, offset=ob.offset,
                                ap=[ob.ap[0], [64, 8], [8, 8], [1, 8]])
                            nc.vector.tensor_tensor(out=obl, in0=psl, in1=xsl,
                                                    op=mybir.AluOpType.add)
                            nc.sync.dma_start(out=out_d[oc, :, t, nb],
                                              in_=ob)

    nc.compile()
    return nc


_BUILD_CACHE = {}


def kernel(x, roi_inds, Wq, Wk, Wv, Wc, gn_gamma, gn_beta):
    global LAST_EXEC_NS
    x = np.ascontiguousarray(np.asarray(x, dtype=np.float32))
    roi_inds = np.asarray(roi_inds, dtype=np.int32)
    n, c = x.shape[0], x.shape[1]
    assert (n, c) == (N, C) and x.shape[2:] == (T, H, W)

    order = np.argsort(roi_inds, kind="stable")
    groups = [order[roi_inds[order] == g] for g in range(N_CORES)]
    sizes = [len(g) for g in groups]
    assert max(sizes) <= NPAD, f"group size {max(sizes)} > {NPAD} unsupported"

    scale = 1.0 / np.sqrt(np.float32(C))

    def prep_w(Wt, sc=1.0):
        w = (np.asarray(Wt, np.float32)[:, :, 0] * sc)  # [O, C, 3, 3]
        w = w.transpose(1, 2, 3, 0).reshape(CC, 128, 9, C)
        return np.ascontiguousarray(w).astype(ml_dtypes.bfloat16)

    w_arrs = {
        "wq": prep_w(Wq, scale), "wk": prep_w(Wk), "wv": prep_w(Wv),
        "wc": prep_w(Wc),
    }
    gamma = np.ascontiguousarray(np.asarray(gn_gamma, np.float32))
    beta = np.ascontiguousarray(np.asarray(gn_beta, np.float32))

    in_maps = []
    for g in range(N_CORES):
        idx = groups[g]
        xg = np.zeros((NPAD, C, T, H, W), np.float32)
        xg[:sizes[g]] = x[idx]
        xcm = xg.transpose(1, 0, 2, 3, 4).reshape(CC, 128, NPAD, T, H, W)
        xpp = np.zeros((CC, 128, NPAD, T, 10, 10), np.float32)
        xpp[..., 1:9, 1:9] = xcm
        xpp = xpp.reshape(CC, 128, NB, 8, T, 100)
        xp = np.ascontiguousarray(
            xpp.transpose(0, 1, 2, 4, 5, 3)).astype(ml_dtypes.bfloat16)
        # mask_big [120,120]: block-diagonal (3 locs), cols >= size -> -1e30
        mb = np.full((120, 120), -1e30, np.float32)
        for b in range(3):
            mb[b * NPAD:(b + 1) * NPAD, b * NPAD:b * NPAD + sizes[g]] = 0.0
        m = {"xp": xp, "maskb": mb, "gamma": gamma, "beta": beta}
        m.update(w_arrs)
        in_maps.append(m)

    if "nc" not in _BUILD_CACHE:
        _BUILD_CACHE["nc"] = _build()
    nc = _BUILD_CACHE["nc"]

    trace = bool(int(os.environ.get("ACAR_TRACE", "0")))
    if trace:
        _install_ntff_hook()
    res = run_bass_kernel_spmd(nc, in_maps, core_ids=list(range(N_CORES)),
                               trace=trace)
    LAST_EXEC_NS = res.exec_time_ns

    out = np.empty((N, C, T, H, W), np.float32)
    for g in range(N_CORES):
        og = res.results[g]["out"]  # [CC, 128, T, NB, (hw, n8)]
        og = og.reshape(C, T, NB, HW, 8)
        og = og.transpose(2, 4, 0, 1, 3).reshape(NPAD, C, T, H, W)
        out[groups[g]] = og[:sizes[g]]
    return out


# revision 9
# speedup vs baseline: 1.0413x; 1.0175x over previous
"""ACAR head (grouped ROI attention) Trainium2 Bass kernel.

Strategy: data-parallel over ROI groups (8 groups -> 8 cores, padded to
npad=40 rows). Attention never crosses groups => no inter-core comm.

v2 restructure vs baseline:
 - attention output computed transposed (avT = v^T @ p^T) so virt lives in
   [c, t, n, loc] layout; kills all phase-C PE transposes and the fp32 DRAM
   spill of virt (kept resident in SBUF as bf16).
 - GroupNorm stats via per-(c,n) loc-reductions + ones-matmul partition fold.
 - softmax packed 3 locations per PE op (npad=40 geometry, M=N=120).
 - residual taken from the bf16 padded x tiles (drops the fp32 xr input).
"""

import os
import sys
import types

sys.path.insert(0, "/opt/trn_rl_repo")

import numpy as np
import ml_dtypes


def _install_ntff_hook():
    try:
        import antenv  # noqa: F401
        from trn_agent_boot.trn_boot import _ntff_profile_via_ctypes

        hook = _ntff_profile_via_ctypes("/opt/axon/libaxon_pjrt.so")
        if hook is None:
            return False
        mod = types.ModuleType("antenv.axon_hooks")
        mod.get_axon_ntff_profile_hook = lambda: hook
        mod.set_axon_ntff_profile_hook = lambda h: None
        sys.modules["antenv.axon_hooks"] = mod
        return True
    except Exception:
        return False


import concourse.bass as bass
import concourse.bacc as bacc
import concourse.tile as tile
from concourse import mybir
from concourse.bass_utils import run_bass_kernel_spmd
from concourse.masks import make_identity

F32 = mybir.dt.float32
BF16 = mybir.dt.bfloat16

N_CORES = 8
N, C, T, H, W = 256, 512, 4, 8, 8
HW = H * W
CC = C // 128
NPAD = 40              # group row pad (multiple of 8, >= max group size)
NB = NPAD // 8
GN_EPS = 1e-5
NELEM = C * T * HW     # per-sample GN element count

# location groups: 20 groups of 3 + 2 groups of 2 (3*40=120 <= 128 partitions)
GROUPS = [(3 * g, 3) for g in range(20)] + [(60, 2), (62, 2)]

LAST_EXEC_NS = None
USE_DMA_T = bool(int(os.environ.get("ACAR_DMA_T", "1")))


def _build():
    nc = bacc.Bacc("TRN2", target_bir_lowering=False, debug=False,
                   num_devices=N_CORES)

    xp_d = nc.dram_tensor("xp", [CC, 128, NB, T, 100, 8], BF16,
                          kind="ExternalInput").ap()
    wq_d = nc.dram_tensor("wq", [CC, 128, 9, C], BF16, kind="ExternalInput").ap()
    wk_d = nc.dram_tensor("wk", [CC, 128, 9, C], BF16, kind="ExternalInput").ap()
    wv_d = nc.dram_tensor("wv", [CC, 128, 9, C], BF16, kind="ExternalInput").ap()
    wc_d = nc.dram_tensor("wc", [CC, 128, 9, C], BF16, kind="ExternalInput").ap()
    maskb_d = nc.dram_tensor("maskb", [120, 120], F32, kind="ExternalInput").ap()
    gamma_d = nc.dram_tensor("gamma", [C], F32, kind="ExternalInput").ap()
    beta_d = nc.dram_tensor("beta", [C], F32, kind="ExternalInput").ap()
    out_d = nc.dram_tensor("out", [CC, 128, T, NB, 512], F32,
                           kind="ExternalOutput").ap()

    def bcast_ap(src, n_part, extra):
        return bass.AP(tensor=src.tensor, offset=src.offset,
                       ap=[[0, n_part]] + extra)

    with tile.TileContext(nc) as tc:
        with (
            tc.tile_pool(name="singles", bufs=1) as singles,
            tc.tile_pool(name="dram", bufs=1, space="DRAM") as dpool,
        ):
            ident = singles.tile([128, 128], F32)
            make_identity(nc, ident)
            ident_bf = singles.tile([128, 128], BF16)
            nc.vector.tensor_copy(out=ident_bf, in_=ident)
            mask_b = singles.tile([120, 120], F32)
            nc.sync.dma_start(out=mask_b, in_=maskb_d)
            ones_t = singles.tile([128, 1], F32)
            nc.vector.memset(ones_t, 1.0)
            eps_t = singles.tile([40, 1], F32)
            nc.vector.memset(eps_t, GN_EPS)
            gam = singles.tile([128, CC], F32)
            bet = singles.tile([128, CC], F32)
            for cc in range(CC):
                nc.sync.dma_start(out=gam[:, cc:cc + 1],
                                  in_=gamma_d[cc * 128:(cc + 1) * 128])
                nc.sync.dma_start(out=bet[:, cc:cc + 1],
                                  in_=beta_d[cc * 128:(cc + 1) * 128])

            # persistent across phases: virt [c, t, n, loc] bf16 + GN partials
            virt = [singles.tile([128, T, NPAD, HW], BF16, name=f"virt{cc}")
                    for cc in range(CC)]
            gnsum = [singles.tile([128, T, NPAD], F32, name=f"gnsum{cc}")
                     for cc in range(CC)]
            gnsq = [singles.tile([128, T, NPAD], F32, name=f"gnsq{cc}")
                    for cc in range(CC)]
            gnd = dpool.tile([NPAD, 2], F32)

            # ============ Phase A: conv(q,k,v) + attention per t ============
            with (
                tc.tile_pool(name="wA", bufs=1) as wpool,
                tc.tile_pool(name="xA", bufs=6) as xpool,
                tc.tile_pool(name="qkvA", bufs=1) as qkv,
                tc.tile_pool(name="sbA", bufs=2) as pB,
                tc.tile_pool(name="sqA", bufs=1) as sqpool,
                tc.tile_pool(name="psA", bufs=1, space="PSUM") as psum,
            ):
                for t in range(T):
                    qkv_sb = {}
                    for name, wd in (("q", wq_d), ("k", wk_d), ("v", wv_d)):
                        w_sb = []
                        for cc in range(CC):
                            wt = wpool.tile([128, 9, C], BF16,
                                            tag=f"w{cc}", name=f"w{cc}")
                            nc.sync.dma_start(out=wt, in_=wd[cc])
                            w_sb.append(wt)
                        for cc in range(CC):
                            if name == "v":
                                # flat + 48-col pad so 128-wide transpose
                                # slices at l0*NPAD stay in bounds
                                vt_ = qkv.tile([128, HW * NPAD + 48], BF16,
                                               tag=f"{name}{cc}",
                                               name=f"{name}{cc}")
                                nc.vector.memset(vt_[:, HW * NPAD:], 0.0)
                                qkv_sb[(name, cc)] = vt_
                            else:
                                qkv_sb[(name, cc)] = qkv.tile(
                                    [128, HW, NPAD], BF16,
                                    tag=f"{name}{cc}", name=f"{name}{cc}")
                        copy_i = 0
                        for nb in range(NB):
                            xs = []
                            for cc in range(CC):
                                xt = xpool.tile([128, 100, 8], BF16, tag="x")
                                nc.sync.dma_start(out=xt, in_=xp_d[cc, :, nb, t])
                                xs.append(xt)
                            for oc in range(4):
                                ps = psum.tile([128, 512], F32, tag="cps",
                                               bufs=2)
                                for cc in range(CC):
                                    for s in range(9):
                                        dh, dw = s // 3, s % 3
                                        xt = xs[cc]
                                        rhs = bass.AP(
                                            tensor=xt.tensor,
                                            offset=xt.offset + (dh * 10 + dw) * 8,
                                            ap=[xt.ap[0], [80, 8], [8, 8],
                                                [1, 8]],
                                        )
                                        nc.tensor.matmul(
                                            ps,
                                            lhsT=w_sb[cc][:, s,
                                                          oc * 128:(oc + 1) * 128],
                                            rhs=rhs,
                                            start=(cc == 0 and s == 0),
                                            stop=(cc == CC - 1 and s == 8),
                                        )
                                # copy PSUM [oc, (hw, n8)] -> tile [oc, hw, n]
                                tl = qkv_sb[(name, oc)]
                                src = bass.AP(tensor=ps.tensor, offset=ps.offset,
                                              ap=[ps.ap[0], [8, HW], [1, 8]])
                                dst = bass.AP(tensor=tl.tensor,
                                              offset=tl.offset + nb * 8,
                                              ap=[tl.ap[0], [NPAD, HW], [1, 8]])
                                if copy_i % 2 == 0:
                                    nc.vector.tensor_copy(out=dst, in_=src)
                                else:
                                    nc.scalar.copy(out=dst, in_=src)
                                copy_i += 1

                    # ---- attention for this t (two passes) ----
                    # pass 1: scores + softmax -> e tiles (PE dense early)
                    e_tiles = []
                    for gi, (l0, gl) in enumerate(GROUPS):
                        m = gl * NPAD
                        s_ps = psum.tile([120, 120], F32, tag="s", bufs=2)
                        for cc in range(CC):
                            nc.tensor.matmul(
                                s_ps[0:m, 0:m],
                                lhsT=qkv_sb[("q", cc)][:, l0:l0 + gl, :],
                                rhs=qkv_sb[("k", cc)][:, l0:l0 + gl, :],
                                start=(cc == 0), stop=(cc == CC - 1),
                            )
                        sm = pB.tile([120, 120], F32, tag="sm")
                        nc.vector.tensor_tensor(
                            out=sm[0:m, 0:m], in0=s_ps[0:m, 0:m],
                            in1=mask_b[0:m, 0:m], op=mybir.AluOpType.add)
                        nm = pB.tile([120, 1], F32, tag="nm")
                        nc.vector.tensor_reduce(
                            out=nm[0:m], in_=sm[0:m, 0:m],
                            axis=mybir.AxisListType.X,
                            op=mybir.AluOpType.max, negate=True)
                        dsum = pB.tile([120, 1], F32, tag="d")
                        e_sb = pB.tile([120, 120], BF16, tag="e", bufs=22)
                        nc.scalar.activation(
                            out=e_sb[0:m, 0:m], in_=sm[0:m, 0:m],
                            func=mybir.ActivationFunctionType.Exp,
                            bias=nm[0:m], scale=1.0, accum_out=dsum[0:m])
                        rr = pB.tile([120, 1], F32, tag="r")
                        nc.vector.reciprocal(out=rr[0:m], in_=dsum[0:m])
                        nc.vector.tensor_scalar_mul(e_sb[0:m, 0:m],
                                                    e_sb[0:m, 0:m], rr[0:m])
                        e_tiles.append(e_sb)
                    # pass 2: transpose + AV (latency-tolerant; conv of next t
                    # can overlap on the PE)
                    for gi, (l0, gl) in enumerate(GROUPS):
                        m = gl * NPAD
                        e_sb = e_tiles[gi]
                        et_ps = psum.tile([120, 120], BF16, tag="et", bufs=1)
                        nc.tensor.transpose(et_ps[0:m, 0:m], e_sb[0:m, 0:m],
                                            ident_bf[0:m, 0:m])
                        et = pB.tile([120, 120], BF16, tag="ets")
                        nc.scalar.copy(out=et[0:m, 0:m], in_=et_ps[0:m, 0:m])

                        for cc in range(CC):
                            v3 = pB.tile([128, 128], BF16, tag=f"v3_{cc}")
                            vtile = qkv_sb[("v", cc)]
                            if USE_DMA_T:
                                vsl = bass.AP(
                                    tensor=vtile.tensor,
                                    offset=vtile.offset + l0 * NPAD,
                                    ap=[vtile.ap[0], [1, 128]])
                                nc.sync.dma_start_transpose(
                                    out=v3, in_=vsl)
                            else:
                                vsl = bass.AP(
                                    tensor=vtile.tensor,
                                    offset=vtile.offset + l0 * NPAD,
                                    ap=[vtile.ap[0], [1, m]])
                                v3p = psum.tile([128, 128], BF16,
                                                tag=f"v3p{cc}", bufs=2)
                                nc.tensor.transpose(v3p[0:m], vsl,
                                                    ident_bf)
                                nc.vector.tensor_copy(out=v3[0:m],
                                                      in_=v3p[0:m])
                            av_ps = psum.tile([128, 120], F32,
                                              tag="av", bufs=2)
                            nc.tensor.matmul(av_ps[:, 0:m], lhsT=v3[0:m],
                                             rhs=et[0:m, 0:m],
                                             start=True, stop=True)
                            src_ = bass.AP(tensor=av_ps.tensor,
                                           offset=av_ps.offset,
                                           ap=[av_ps.ap[0], [NPAD, gl],
                                               [1, NPAD]])
                            vt = virt[cc]
                            dst = bass.AP(
                                tensor=vt.tensor,
                                offset=vt.offset + t * NPAD * HW + l0,
                                ap=[vt.ap[0], [1, gl], [HW, NPAD]])
                            nc.scalar.copy(out=dst, in_=src_)

                    # ---- GN partial sums for this t ----
                    for cc in range(CC):
                        vsl = bass.AP(
                            tensor=virt[cc].tensor,
                            offset=virt[cc].offset + t * NPAD * HW,
                            ap=[virt[cc].ap[0], [HW, NPAD], [1, HW]])
                        nc.vector.tensor_reduce(
                            out=gnsum[cc][:, t], in_=vsl,
                            axis=mybir.AxisListType.X, op=mybir.AluOpType.add)
                        for nh in range(2):
                            vslh = bass.AP(
                                tensor=virt[cc].tensor,
                                offset=virt[cc].offset + t * NPAD * HW
                                + nh * (NPAD // 2) * HW,
                                ap=[virt[cc].ap[0], [HW, NPAD // 2], [1, HW]])
                            sq = sqpool.tile([128, NPAD // 2, HW], BF16,
                                             tag="sq")
                            eng = nc.vector if cc % 2 == 0 else nc.gpsimd
                            eng.tensor_tensor(out=sq, in0=vslh, in1=vslh,
                                              op=mybir.AluOpType.mult)
                            nc.vector.tensor_reduce(
                                out=gnsq[cc][:, t,
                                             nh * (NPAD // 2):(nh + 1)
                                             * (NPAD // 2)],
                                in_=sq, axis=mybir.AxisListType.X,
                                op=mybir.AluOpType.add)

                # ---- GN finalize (one psum tile, sequential groups) ----
                gn_ps = psum.tile([NPAD, 1], F32, tag="gns", bufs=1)
                k = 0
                for cc in range(CC):
                    for t in range(T):
                        nc.tensor.matmul(gn_ps, lhsT=gnsum[cc][:, t],
                                         rhs=ones_t, start=(k == 0),
                                         stop=(k == CC * T - 1))
                        k += 1
                inv = 1.0 / NELEM
                mu = pB.tile([NPAD, 1], F32, tag="mu")
                nc.vector.tensor_scalar_mul(mu, gn_ps, inv)
                k = 0
                for cc in range(CC):
                    for t in range(T):
                        nc.tensor.matmul(gn_ps, lhsT=gnsq[cc][:, t],
                                         rhs=ones_t, start=(k == 0),
                                         stop=(k == CC * T - 1))
                        k += 1
                e2 = pB.tile([NPAD, 1], F32, tag="e2")
                nc.vector.tensor_scalar_mul(e2, gn_ps, inv)
                mu2 = pB.tile([NPAD, 1], F32, tag="mu2")
                nc.vector.tensor_mul(mu2, mu, mu)
                var = pB.tile([NPAD, 1], F32, tag="var")
                nc.vector.tensor_sub(var, e2, mu2)
                rstd = pB.tile([NPAD, 1], F32, tag="rstd")
                nc.scalar.activation(out=rstd, in_=var,
                                     func=mybir.ActivationFunctionType.Sqrt,
                                     bias=eps_t, scale=1.0)
                nc.vector.reciprocal(out=rstd, in_=rstd)
                murstd = pB.tile([NPAD, 1], F32, tag="murstd")
                nc.vector.tensor_mul(murstd, mu, rstd)
                gpack = pB.tile([NPAD, 2], F32, tag="gpack")
                nc.vector.tensor_copy(out=gpack[:, 0:1], in_=rstd)
                nc.vector.tensor_copy(out=gpack[:, 1:2], in_=murstd)
                nc.sync.dma_start(out=gnd, in_=gpack)

            # broadcast (rstd, mu*rstd) to all partitions: [128, NPAD, 2]
            abn = singles.tile([128, NPAD, 2], F32)
            nc.gpsimd.dma_start(out=abn,
                                in_=bcast_ap(gnd, 128, [[2, NPAD], [1, 2]]))

            # ============ Phase C: GN apply + Wc conv + residual ============
            with (
                tc.tile_pool(name="wC", bufs=1) as wpool,
                tc.tile_pool(name="xC", bufs=8) as xcpool,
                tc.tile_pool(name="padC", bufs=1) as padpool,
                tc.tile_pool(name="ioC", bufs=3) as iopool,
                tc.tile_pool(name="psC", bufs=1, space="PSUM") as psC,
            ):
                wc_sb = []
                for cc in range(CC):
                    wt = wpool.tile([128, 9, C], BF16, tag=f"wc{cc}")
                    nc.sync.dma_start(out=wt, in_=wc_d[cc])
                    wc_sb.append(wt)
                zeros1 = iopool.tile([128, 1], F32, tag="z", bufs=1)
                nc.vector.memset(zeros1, 0.0)
                vpads = []
                for par in range(2):
                    row = []
                    for cc in range(CC):
                        vp = padpool.tile([128, 100, 8], BF16,
                                          tag=f"vp{par}_{cc}")
                        nc.vector.tensor_copy(
                            out=vp,
                            in_=bass.AP(tensor=zeros1.tensor,
                                        offset=zeros1.offset,
                                        ap=[zeros1.ap[0], [0, 100], [0, 8]]))
                        row.append(vp)
                    vpads.append(row)
                it_c = 0
                for t in range(T):
                    # GN apply + relu for this t, in place on virt (bf16)
                    for cc in range(CC):
                        vt = virt[cc]
                        vap = bass.AP(tensor=vt.tensor,
                                      offset=vt.offset + t * NPAD * HW,
                                      ap=[vt.ap[0], [HW, NPAD], [1, HW]])
                        rb = bass.AP(tensor=abn.tensor, offset=abn.offset,
                                     ap=[abn.ap[0], [2, NPAD], [0, HW]])
                        mb = bass.AP(tensor=abn.tensor, offset=abn.offset + 1,
                                     ap=[abn.ap[0], [2, NPAD], [0, HW]])
                        eng = nc.vector if cc % 2 == 0 else nc.gpsimd
                        eng.tensor_tensor(out=vap, in0=vap, in1=rb,
                                          op=mybir.AluOpType.mult)
                        eng.tensor_tensor(out=vap, in0=vap, in1=mb,
                                          op=mybir.AluOpType.subtract)
                        nc.scalar.activation(
                            out=vap, in_=vap,
                            func=mybir.ActivationFunctionType.Relu,
                            bias=bet[:, cc:cc + 1], scale=gam[:, cc:cc + 1])
                    for nb in range(NB):
                        par = it_c % 2
                        it_c += 1
                        xs = []
                        for cc in range(CC):
                            xt = xcpool.tile([128, 100, 8], BF16, tag="xc")
                            nc.sync.dma_start(out=xt, in_=xp_d[cc, :, nb, t])
                            xs.append(xt)
                        for cc in range(CC):
                            vt = virt[cc]
                            src = bass.AP(
                                tensor=vt.tensor,
                                offset=vt.offset + t * NPAD * HW + nb * 8 * HW,
                                ap=[vt.ap[0], [8, 8], [1, 8], [HW, 8]])
                            vp = vpads[par][cc]
                            dst = bass.AP(
                                tensor=vp.tensor,
                                offset=vp.offset + 11 * 8,
                                ap=[vp.ap[0], [80, 8], [8, 8], [1, 8]])
                            eng = nc.vector if cc % 2 == 0 else nc.gpsimd
                            eng.tensor_copy(out=dst, in_=src)
                        for oc in range(4):
                            ps = psC.tile([128, 512], F32, tag="cps", bufs=4)
                            for cc in range(CC):
                                for s in range(9):
                                    dh, dw = s // 3, s % 3
                                    vp = vpads[par][cc]
                                    rhs = bass.AP(
                                        tensor=vp.tensor,
                                        offset=vp.offset + (dh * 10 + dw) * 8,
                                        ap=[vp.ap[0], [80, 8], [8, 8],
                                            [1, 8]])
                                    nc.tensor.matmul(
                                        ps,
                                        lhsT=wc_sb[cc][:, s,
                                                       oc * 128:(oc + 1) * 128],
                                        rhs=rhs,
                                        start=(cc == 0 and s == 0),
                                        stop=(cc == CC - 1 and s == 8))
                            xsl = bass.AP(
                                tensor=xs[oc].tensor,
                                offset=xs[oc].offset + 11 * 8,
                                ap=[xs[oc].ap[0], [80, 8], [8, 8], [1, 8]])
                            psl = bass.AP(
                                tensor=ps.tensor, offset=ps.offset,
                                ap=[ps.ap[0], [64, 8], [8, 8], [1, 8]])
                            ob = iopool.tile([128, 512], F32, tag="cob")
                            obl = bass.AP(
                                tensor=ob.tensor, offset=ob.offset,
                                ap=[ob.ap[0], [64, 8], [8, 8], [1, 8]])
                            nc.vector.tensor_tensor(out=obl, in0=psl, in1=xsl,
                                                    op=mybir.AluOpType.add)
                            nc.sync.dma_start(out=out_d[oc, :, t, nb],
                                              in_=ob)

    nc.compile()
    return nc


_BUILD_CACHE = {}


def kernel(x, roi_inds, Wq, Wk, Wv, Wc, gn_gamma, gn_beta):
    global LAST_EXEC_NS
    x = np.ascontiguousarray(np.asarray(x, dtype=np.float32))
    roi_inds = np.asarray(roi_inds, dtype=np.int32)
    n, c = x.shape[0], x.shape[1]
    assert (n, c) == (N, C) and x.shape[2:] == (T, H, W)

    order = np.argsort(roi_inds, kind="stable")
    groups = [order[roi_inds[order] == g] for g in range(N_CORES)]
    sizes = [len(g) for g in groups]
    assert max(sizes) <= NPAD, f"group size {max(sizes)} > {NPAD} unsupported"

    scale = 1.0 / np.sqrt(np.float32(C))

    def prep_w(Wt, sc=1.0):
        w = (np.asarray(Wt, np.float32)[:, :, 0] * sc)  # [O, C, 3, 3]
        w = w.transpose(1, 2, 3, 0).reshape(CC, 128, 9, C)
        return np.ascontiguousarray(w).astype(ml_dtypes.bfloat16)

    w_arrs = {
        "wq": prep_w(Wq, scale), "wk": prep_w(Wk), "wv": prep_w(Wv),
        "wc": prep_w(Wc),
    }
    gamma = np.ascontiguousarray(np.asarray(gn_gamma, np.float32))
    beta = np.ascontiguousarray(np.asarray(gn_beta, np.float32))

    in_maps = []
    for g in range(N_CORES):
        idx = groups[g]
        xg = np.zeros((NPAD, C, T, H, W), np.float32)
        xg[:sizes[g]] = x[idx]
        xcm = xg.transpose(1, 0, 2, 3, 4).reshape(CC, 128, NPAD, T, H, W)
        xpp = np.zeros((CC, 128, NPAD, T, 10, 10), np.float32)
        xpp[..., 1:9, 1:9] = xcm
        xpp = xpp.reshape(CC, 128, NB, 8, T, 100)
        xp = np.ascontiguousarray(
            xpp.transpose(0, 1, 2, 4, 5, 3)).astype(ml_dtypes.bfloat16)
        # mask_big [120,120]: block-diagonal (3 locs), cols >= size -> -1e30
        mb = np.full((120, 120), -1e30, np.float32)
        for b in range(3):
            mb[b * NPAD:(b + 1) * NPAD, b * NPAD:b * NPAD + sizes[g]] = 0.0
        m = {"xp": xp, "maskb": mb, "gamma": gamma, "beta": beta}
        m.update(w_arrs)
        in_maps.append(m)

    if "nc" not in _BUILD_CACHE:
        _BUILD_CACHE["nc"] = _build()
    nc = _BUILD_CACHE["nc"]

    trace = bool(int(os.environ.get("ACAR_TRACE", "0")))
    if trace:
        _install_ntff_hook()
    res = run_bass_kernel_spmd(nc, in_maps, core_ids=list(range(N_CORES)),
                               trace=trace)
    LAST_EXEC_NS = res.exec_time_ns

    out = np.empty((N, C, T, H, W), np.float32)
    for g in range(N_CORES):
        og = res.results[g]["out"]  # [CC, 128, T, NB, (hw, n8)]
        og = og.reshape(C, T, NB, HW, 8)
        og = og.transpose(2, 4, 0, 1, 3).reshape(NPAD, C, T, H, W)
        out[groups[g]] = og[:sizes[g]]
    return out


# revision 12
# speedup vs baseline: 1.1439x; 1.0986x over previous
"""ACAR head (grouped ROI attention) Trainium2 Bass kernel.

Strategy: data-parallel over ROI groups (8 groups -> 8 cores, padded to
npad=40 rows). Attention never crosses groups => no inter-core comm.

v2 restructure vs baseline:
 - attention output computed transposed (avT = v^T @ p^T) so virt lives in
   [c, t, n, loc] layout; kills all phase-C PE transposes and the fp32 DRAM
   spill of virt (kept resident in SBUF as bf16).
 - GroupNorm stats via per-(c,n) loc-reductions + ones-matmul partition fold.
 - softmax packed 3 locations per PE op (npad=40 geometry, M=N=120).
 - residual taken from the bf16 padded x tiles (drops the fp32 xr input).
"""

import os
import sys
import types

sys.path.insert(0, "/opt/trn_rl_repo")

import numpy as np
import ml_dtypes


def _install_ntff_hook():
    try:
        import antenv  # noqa: F401
        from trn_agent_boot.trn_boot import _ntff_profile_via_ctypes

        hook = _ntff_profile_via_ctypes("/opt/axon/libaxon_pjrt.so")
        if hook is None:
            return False
        mod = types.ModuleType("antenv.axon_hooks")
        mod.get_axon_ntff_profile_hook = lambda: hook
        mod.set_axon_ntff_profile_hook = lambda h: None
        sys.modules["antenv.axon_hooks"] = mod
        return True
    except Exception:
        return False


import concourse.bass as bass
import concourse.bacc as bacc
import concourse.tile as tile
from concourse import mybir
from concourse.bass_utils import run_bass_kernel_spmd
from concourse.masks import make_identity

F32 = mybir.dt.float32
BF16 = mybir.dt.bfloat16

N_CORES = 8
N, C, T, H, W = 256, 512, 4, 8, 8
HW = H * W
CC = C // 128
NPAD = 40              # group row pad (multiple of 8, >= max group size)
NB = NPAD // 8
GN_EPS = 1e-5
NELEM = C * T * HW     # per-sample GN element count

# location groups: 20 groups of 3 + 2 groups of 2 (3*40=120 <= 128 partitions)
GROUPS = [(3 * g, 3) for g in range(20)] + [(60, 2), (62, 2)]

LAST_EXEC_NS = None
USE_DMA_T = bool(int(os.environ.get("ACAR_DMA_T", "1")))


def _build():
    nc = bacc.Bacc("TRN2", target_bir_lowering=False, debug=False,
                   num_devices=N_CORES)

    xp_d = nc.dram_tensor("xp", [CC, 128, NB, T, 100, 8], BF16,
                          kind="ExternalInput").ap()
    wq_d = nc.dram_tensor("wq", [CC, 128, 9, C], BF16, kind="ExternalInput").ap()
    wk_d = nc.dram_tensor("wk", [CC, 128, 9, C], BF16, kind="ExternalInput").ap()
    wv_d = nc.dram_tensor("wv", [CC, 128, 9, C], BF16, kind="ExternalInput").ap()
    wc_d = nc.dram_tensor("wc", [CC, 128, 9, C], BF16, kind="ExternalInput").ap()
    maskb_d = nc.dram_tensor("maskb", [120, 120], F32, kind="ExternalInput").ap()
    gamma_d = nc.dram_tensor("gamma", [C], F32, kind="ExternalInput").ap()
    beta_d = nc.dram_tensor("beta", [C], F32, kind="ExternalInput").ap()
    out_d = nc.dram_tensor("out", [CC, 128, T, NB, 512], F32,
                           kind="ExternalOutput").ap()

    def bcast_ap(src, n_part, extra):
        return bass.AP(tensor=src.tensor, offset=src.offset,
                       ap=[[0, n_part]] + extra)

    with tile.TileContext(nc) as tc:
        with (
            tc.tile_pool(name="singles", bufs=1) as singles,
            tc.tile_pool(name="dram", bufs=1, space="DRAM") as dpool,
        ):
            ident = singles.tile([128, 128], F32)
            make_identity(nc, ident)
            ident_bf = singles.tile([128, 128], BF16)
            nc.vector.tensor_copy(out=ident_bf, in_=ident)
            mask_b = singles.tile([120, 120], F32)
            nc.sync.dma_start(out=mask_b, in_=maskb_d)
            ones_b = singles.tile([128, 1], BF16)
            nc.vector.memset(ones_b, 1.0)
            eps_t = singles.tile([40, 1], F32)
            nc.vector.memset(eps_t, GN_EPS)
            gam = singles.tile([128, CC], F32)
            bet = singles.tile([128, CC], F32)
            for cc in range(CC):
                nc.sync.dma_start(out=gam[:, cc:cc + 1],
                                  in_=gamma_d[cc * 128:(cc + 1) * 128])
                nc.sync.dma_start(out=bet[:, cc:cc + 1],
                                  in_=beta_d[cc * 128:(cc + 1) * 128])

            # persistent across phases: virt [c, t, n, loc] bf16 + GN partials
            virt = [singles.tile([128, T, NPAD, HW], BF16, name=f"virt{cc}")
                    for cc in range(CC)]
            gnsum = [singles.tile([128, T, NPAD], F32, name=f"gnsum{cc}")
                     for cc in range(CC)]
            gnsq = [singles.tile([128, T, NPAD], F32, name=f"gnsq{cc}")
                    for cc in range(CC)]
            gnd = dpool.tile([NPAD, 2], F32)

            # ============ Phase A: conv(q,k,v) + attention per t ============
            with (
                tc.tile_pool(name="wA", bufs=1) as wpool,
                tc.tile_pool(name="xA", bufs=6) as xpool,
                tc.tile_pool(name="qkvA", bufs=1) as qkv,
                tc.tile_pool(name="sbA", bufs=2) as pB,
                tc.tile_pool(name="sqA", bufs=1) as sqpool,
                tc.tile_pool(name="psA", bufs=1, space="PSUM") as psum,
            ):
                from collections import deque
                drain = deque()
                est_all = {}
                st_all = {}

                def emit_pass1(qkv_sb, l0, gl):
                    m = gl * NPAD
                    s_ps = psum.tile([120, 120], F32, tag="s", bufs=2)
                    for cc in range(CC):
                        nc.tensor.matmul(
                            s_ps[0:m, 0:m],
                            lhsT=qkv_sb[("q", cc)][:, l0:l0 + gl, :],
                            rhs=qkv_sb[("k", cc)][:, l0:l0 + gl, :],
                            start=(cc == 0), stop=(cc == CC - 1),
                        )
                    sm = pB.tile([120, 120], F32, tag="sm")
                    nc.vector.tensor_tensor(
                        out=sm[0:m, 0:m], in0=s_ps[0:m, 0:m],
                        in1=mask_b[0:m, 0:m], op=mybir.AluOpType.add)
                    nm = pB.tile([120, 1], F32, tag="nm")
                    nc.vector.tensor_reduce(
                        out=nm[0:m], in_=sm[0:m, 0:m],
                        axis=mybir.AxisListType.X,
                        op=mybir.AluOpType.max, negate=True)
                    dsum = pB.tile([120, 1], F32, tag="d")
                    e_sb = pB.tile([120, 120], BF16, tag="e", bufs=22)
                    nc.scalar.activation(
                        out=e_sb[0:m, 0:m], in_=sm[0:m, 0:m],
                        func=mybir.ActivationFunctionType.Exp,
                        bias=nm[0:m], scale=1.0, accum_out=dsum[0:m])
                    rr = pB.tile([120, 1], F32, tag="r")
                    nc.vector.reciprocal(out=rr[0:m], in_=dsum[0:m])
                    nc.vector.tensor_scalar_mul(e_sb[0:m, 0:m],
                                                e_sb[0:m, 0:m], rr[0:m])
                    return e_sb

                def emit_pass2_s1(qkv_sb, e_sb, l0, gl):
                    m = gl * NPAD
                    et_ps = psum.tile([120, 120], BF16, tag="et", bufs=1)
                    nc.tensor.transpose(et_ps[0:m, 0:m], e_sb[0:m, 0:m],
                                        ident_bf[0:m, 0:m])
                    et = pB.tile([120, 120], BF16, tag="ets")
                    nc.scalar.copy(out=et[0:m, 0:m], in_=et_ps[0:m, 0:m])
                    v3s = []
                    for cc in range(CC):
                        v3 = pB.tile([128, 128], BF16, tag=f"v3_{cc}")
                        vtile = qkv_sb[("v", cc)]
                        if USE_DMA_T:
                            vsl = bass.AP(
                                tensor=vtile.tensor,
                                offset=vtile.offset + l0 * NPAD,
                                ap=[vtile.ap[0], [1, 128]])
                            nc.sync.dma_start_transpose(out=v3, in_=vsl)
                        else:
                            vsl = bass.AP(
                                tensor=vtile.tensor,
                                offset=vtile.offset + l0 * NPAD,
                                ap=[vtile.ap[0], [1, m]])
                            v3p = psum.tile([128, 128], BF16,
                                            tag=f"v3p{cc}", bufs=2)
                            nc.tensor.transpose(v3p[0:m], vsl, ident_bf)
                            nc.vector.tensor_copy(out=v3[0:m], in_=v3p[0:m])
                        v3s.append(v3)
                    return et, v3s

                def emit_pass2_s2(t, et, v3s, l0, gl):
                    m = gl * NPAD
                    for cc in range(CC):
                        av_ps = psum.tile([128, 120], F32, tag="av", bufs=2)
                        nc.tensor.matmul(av_ps[:, 0:m], lhsT=v3s[cc][0:m],
                                         rhs=et[0:m, 0:m],
                                         start=True, stop=True)
                        src_ = bass.AP(tensor=av_ps.tensor,
                                       offset=av_ps.offset,
                                       ap=[av_ps.ap[0], [NPAD, gl],
                                           [1, NPAD]])
                        vt = virt[cc]
                        dst = bass.AP(
                            tensor=vt.tensor,
                            offset=vt.offset + t * NPAD * HW + l0,
                            ap=[vt.ap[0], [1, gl], [HW, NPAD]])
                        nc.scalar.copy(out=dst, in_=src_)

                def emit_gnred(t, cc):
                    vsl = bass.AP(
                        tensor=virt[cc].tensor,
                        offset=virt[cc].offset + t * NPAD * HW,
                        ap=[virt[cc].ap[0], [HW, NPAD], [1, HW]])
                    nc.vector.tensor_reduce(
                        out=gnsum[cc][:, t], in_=vsl,
                        axis=mybir.AxisListType.X, op=mybir.AluOpType.add)
                    for nh in range(2):
                        vslh = bass.AP(
                            tensor=virt[cc].tensor,
                            offset=virt[cc].offset + t * NPAD * HW
                            + nh * (NPAD // 2) * HW,
                            ap=[virt[cc].ap[0], [HW, NPAD // 2], [1, HW]])
                        sq = sqpool.tile([128, NPAD // 2, HW], BF16,
                                         tag="sq")
                        eng = nc.vector if cc % 2 == 0 else nc.gpsimd
                        eng.tensor_tensor(out=sq, in0=vslh, in1=vslh,
                                          op=mybir.AluOpType.mult)
                        nc.vector.tensor_reduce(
                            out=gnsq[cc][:, t,
                                         nh * (NPAD // 2):(nh + 1)
                                         * (NPAD // 2)],
                            in_=sq, axis=mybir.AxisListType.X,
                            op=mybir.AluOpType.add)

                def pump(nmax=1):
                    for _ in range(nmax):
                        if drain:
                            drain.popleft()()

                for t in range(T):
                    qkv_sb = {}
                    for name, wd in (("q", wq_d), ("k", wk_d), ("v", wv_d)):
                        w_sb = []
                        for cc in range(CC):
                            wt = wpool.tile([128, 9, C], BF16,
                                            tag=f"w{cc}", name=f"w{cc}")
                            nc.sync.dma_start(out=wt, in_=wd[cc])
                            w_sb.append(wt)
                        for cc in range(CC):
                            if name == "v":
                                vt_ = qkv.tile([128, HW * NPAD + 48], BF16,
                                               tag=f"{name}{cc}",
                                               name=f"{name}{cc}")
                                nc.vector.memset(vt_[:, HW * NPAD:], 0.0)
                                qkv_sb[(name, cc)] = vt_
                            else:
                                qkv_sb[(name, cc)] = qkv.tile(
                                    [128, HW, NPAD], BF16,
                                    tag=f"{name}{cc}", name=f"{name}{cc}")
                        if name == "v":
                            # queue pass1 for this t (reads only q,k)
                            def mk1(qsb, l0_, gl_, t_):
                                def f():
                                    est_all[(t_, l0_)] = emit_pass1(
                                        qsb, l0_, gl_)
                                return f
                            for (l0, gl) in GROUPS:
                                drain.append(mk1(qkv_sb, l0, gl, t))
                        copy_i = 0
                        for nb in range(NB):
                            xs = []
                            for cc in range(CC):
                                xt = xpool.tile([128, 100, 8], BF16, tag="x")
                                nc.sync.dma_start(out=xt, in_=xp_d[cc, :, nb, t])
                                xs.append(xt)
                            for oc in range(4):
                                ps = psum.tile([128, 512], F32, tag="cps",
                                               bufs=2)
                                for cc in range(CC):
                                    for s in range(9):
                                        dh, dw = s // 3, s % 3
                                        xt = xs[cc]
                                        rhs = bass.AP(
                                            tensor=xt.tensor,
                                            offset=xt.offset + (dh * 10 + dw) * 8,
                                            ap=[xt.ap[0], [80, 8], [8, 8],
                                                [1, 8]],
                                        )
                                        nc.tensor.matmul(
                                            ps,
                                            lhsT=w_sb[cc][:, s,
                                                          oc * 128:(oc + 1) * 128],
                                            rhs=rhs,
                                            start=(cc == 0 and s == 0),
                                            stop=(cc == CC - 1 and s == 8),
                                        )
                                tl = qkv_sb[(name, oc)]
                                src = bass.AP(tensor=ps.tensor, offset=ps.offset,
                                              ap=[ps.ap[0], [8, HW], [1, 8]])
                                dst = bass.AP(tensor=tl.tensor,
                                              offset=tl.offset + nb * 8,
                                              ap=[tl.ap[0], [NPAD, HW], [1, 8]])
                                if copy_i % 2 == 0:
                                    nc.vector.tensor_copy(out=dst, in_=src)
                                else:
                                    nc.scalar.copy(out=dst, in_=src)
                                copy_i += 1
                                pump(2 if len(drain) > 40 else 1)

                    # queue pass2 (two stages, 1-group spacing) + GN reduces
                    def mk2a(qsb, t_, l0_, gl_):
                        def f():
                            e = est_all[(t_, l0_)]
                            st_all[(t_, l0_)] = emit_pass2_s1(
                                qsb, e, l0_, gl_)
                        return f

                    def mk2b(t_, l0_, gl_):
                        def f():
                            et, v3s = st_all[(t_, l0_)]
                            emit_pass2_s2(t_, et, v3s, l0_, gl_)
                        return f

                    prev = None
                    for (l0, gl) in GROUPS:
                        drain.append(mk2a(qkv_sb, t, l0, gl))
                        if prev is not None:
                            drain.append(mk2b(t, *prev))
                        prev = (l0, gl)
                    drain.append(mk2b(t, *prev))

                    def mkg(t_, cc_):
                        return lambda: emit_gnred(t_, cc_)
                    for cc in range(CC):
                        drain.append(mkg(t, cc))

                # flush remaining (pass2 + gn of last t)
                while drain:
                    drain.popleft()()

                # ---- GN finalize (bf16 partials, one psum tile) ----
                gnbs = []
                for cc in range(CC):
                    gnb = pB.tile([128, 2, T, NPAD], BF16, tag="gnb", bufs=4)
                    nc.vector.tensor_copy(out=gnb[:, 0], in_=gnsum[cc])
                    nc.vector.tensor_copy(out=gnb[:, 1], in_=gnsq[cc])
                    gnbs.append(gnb)
                gn_ps = psum.tile([NPAD, 1], F32, tag="gns", bufs=1)
                k = 0
                for cc in range(CC):
                    for t in range(T):
                        nc.tensor.matmul(gn_ps, lhsT=gnbs[cc][:, 0, t],
                                         rhs=ones_b, start=(k == 0),
                                         stop=(k == CC * T - 1))
                        k += 1
                inv = 1.0 / NELEM
                mu = pB.tile([NPAD, 1], F32, tag="mu")
                nc.vector.tensor_scalar_mul(mu, gn_ps, inv)
                k = 0
                for cc in range(CC):
                    for t in range(T):
                        nc.tensor.matmul(gn_ps, lhsT=gnbs[cc][:, 1, t],
                                         rhs=ones_b, start=(k == 0),
                                         stop=(k == CC * T - 1))
                        k += 1
                e2 = pB.tile([NPAD, 1], F32, tag="e2")
                nc.vector.tensor_scalar_mul(e2, gn_ps, inv)
                mu2 = pB.tile([NPAD, 1], F32, tag="mu2")
                nc.vector.tensor_mul(mu2, mu, mu)
                var = pB.tile([NPAD, 1], F32, tag="var")
                nc.vector.tensor_sub(var, e2, mu2)
                rstd = pB.tile([NPAD, 1], F32, tag="rstd")
                nc.scalar.activation(out=rstd, in_=var,
                                     func=mybir.ActivationFunctionType.Sqrt,
                                     bias=eps_t, scale=1.0)
                nc.vector.reciprocal(out=rstd, in_=rstd)
                murstd = pB.tile([NPAD, 1], F32, tag="murstd")
                nc.vector.tensor_mul(murstd, mu, rstd)
                gpack = pB.tile([NPAD, 2], F32, tag="gpack")
                nc.vector.tensor_copy(out=gpack[:, 0:1], in_=rstd)
                nc.vector.tensor_copy(out=gpack[:, 1:2], in_=murstd)
                nc.sync.dma_start(out=gnd, in_=gpack)

            # broadcast (rstd, mu*rstd) to all partitions: [128, NPAD, 2]
            abn = singles.tile([128, NPAD, 2], F32)
            nc.gpsimd.dma_start(out=abn,
                                in_=bcast_ap(gnd, 128, [[2, NPAD], [1, 2]]))

            # ============ Phase C: GN apply + Wc conv + residual ============
            with (
                tc.tile_pool(name="wC", bufs=1) as wpool,
                tc.tile_pool(name="xC", bufs=8) as xcpool,
                tc.tile_pool(name="padC", bufs=1) as padpool,
                tc.tile_pool(name="ioC", bufs=3) as iopool,
                tc.tile_pool(name="psC", bufs=1, space="PSUM") as psC,
            ):
                wc_sb = []
                for cc in range(CC):
                    wt = wpool.tile([128, 9, C], BF16, tag=f"wc{cc}")
                    nc.sync.dma_start(out=wt, in_=wc_d[cc])
                    wc_sb.append(wt)
                zeros1 = iopool.tile([128, 1], F32, tag="z", bufs=1)
                nc.vector.memset(zeros1, 0.0)
                vpads = []
                for par in range(2):
                    row = []
                    for cc in range(CC):
                        vp = padpool.tile([128, 100, 8], BF16,
                                          tag=f"vp{par}_{cc}")
                        nc.vector.tensor_copy(
                            out=vp,
                            in_=bass.AP(tensor=zeros1.tensor,
                                        offset=zeros1.offset,
                                        ap=[zeros1.ap[0], [0, 100], [0, 8]]))
                        row.append(vp)
                    vpads.append(row)
                it_c = 0
                for t in range(T):
                    # GN apply + relu for this t, in place on virt (bf16)
                    for cc in range(CC):
                        vt = virt[cc]
                        vap = bass.AP(tensor=vt.tensor,
                                      offset=vt.offset + t * NPAD * HW,
                                      ap=[vt.ap[0], [HW, NPAD], [1, HW]])
                        rb = bass.AP(tensor=abn.tensor, offset=abn.offset,
                                     ap=[abn.ap[0], [2, NPAD], [0, HW]])
                        mb = bass.AP(tensor=abn.tensor, offset=abn.offset + 1,
                                     ap=[abn.ap[0], [2, NPAD], [0, HW]])
                        eng = nc.vector if cc % 2 == 0 else nc.gpsimd
                        eng.tensor_tensor(out=vap, in0=vap, in1=rb,
                                          op=mybir.AluOpType.mult)
                        eng.tensor_tensor(out=vap, in0=vap, in1=mb,
                                          op=mybir.AluOpType.subtract)
                        nc.scalar.activation(
                            out=vap, in_=vap,
                            func=mybir.ActivationFunctionType.Relu,
                            bias=bet[:, cc:cc + 1], scale=gam[:, cc:cc + 1])
                    for nb in range(NB):
                        par = it_c % 2
                        it_c += 1
                        xs = []
                        for cc in range(CC):
                            xt = xcpool.tile([128, 100, 8], BF16, tag="xc")
                            nc.sync.dma_start(out=xt, in_=xp_d[cc, :, nb, t])
                            xs.append(xt)
                        for cc in range(CC):
                            vt = virt[cc]
                            src = bass.AP(
                                tensor=vt.tensor,
                                offset=vt.offset + t * NPAD * HW + nb * 8 * HW,
                                ap=[vt.ap[0], [8, 8], [1, 8], [HW, 8]])
                            vp = vpads[par][cc]
                            dst = bass.AP(
                                tensor=vp.tensor,
                                offset=vp.offset + 11 * 8,
                                ap=[vp.ap[0], [80, 8], [8, 8], [1, 8]])
                            eng = nc.vector if cc % 2 == 0 else nc.gpsimd
                            eng.tensor_copy(out=dst, in_=src)
                        for oc in range(4):
                            ps = psC.tile([128, 512], F32, tag="cps", bufs=4)
                            for cc in range(CC):
                                for s in range(9):
                                    dh, dw = s // 3, s % 3
                                    vp = vpads[par][cc]
                                    rhs = bass.AP(
                                        tensor=vp.tensor,
                                        offset=vp.offset + (dh * 10 + dw) * 8,
                                        ap=[vp.ap[0], [80, 8], [8, 8],
                                            [1, 8]])
                                    nc.tensor.matmul(
                                        ps,
                                        lhsT=wc_sb[cc][:, s,
                                                       oc * 128:(oc + 1) * 128],
                                        rhs=rhs,
                                        start=(cc == 0 and s == 0),
                                        stop=(cc == CC - 1 and s == 8))
                            xsl = bass.AP(
                                tensor=xs[oc].tensor,
                                offset=xs[oc].offset + 11 * 8,
                                ap=[xs[oc].ap[0], [80, 8], [8, 8], [1, 8]])
                            psl = bass.AP(
                                tensor=ps.tensor, offset=ps.offset,
                                ap=[ps.ap[0], [64, 8], [8, 8], [1, 8]])
                            ob = iopool.tile([128, 512], F32, tag="cob")
                            obl = bass.AP(
                                tensor=ob.tensor, offset=ob.offset,
                                ap=[ob.ap[0], [64, 8], [8, 8], [1, 8]])
                            nc.vector.tensor_tensor(out=obl, in0=psl, in1=xsl,
                                                    op=mybir.AluOpType.add)
                            nc.sync.dma_start(out=out_d[oc, :, t, nb],
                                              in_=ob)

    nc.compile()
    return nc


_BUILD_CACHE = {}


def kernel(x, roi_inds, Wq, Wk, Wv, Wc, gn_gamma, gn_beta):
    global LAST_EXEC_NS
    x = np.ascontiguousarray(np.asarray(x, dtype=np.float32))
    roi_inds = np.asarray(roi_inds, dtype=np.int32)
    n, c = x.shape[0], x.shape[1]
    assert (n, c) == (N, C) and x.shape[2:] == (T, H, W)

    order = np.argsort(roi_inds, kind="stable")
    groups = [order[roi_inds[order] == g] for g in range(N_CORES)]
    sizes = [len(g) for g in groups]
    assert max(sizes) <= NPAD, f"group size {max(sizes)} > {NPAD} unsupported"

    scale = 1.0 / np.sqrt(np.float32(C))

    def prep_w(Wt, sc=1.0):
        w = (np.asarray(Wt, np.float32)[:, :, 0] * sc)  # [O, C, 3, 3]
        w = w.transpose(1, 2, 3, 0).reshape(CC, 128, 9, C)
        return np.ascontiguousarray(w).astype(ml_dtypes.bfloat16)

    w_arrs = {
        "wq": prep_w(Wq, scale), "wk": prep_w(Wk), "wv": prep_w(Wv),
        "wc": prep_w(Wc),
    }
    gamma = np.ascontiguousarray(np.asarray(gn_gamma, np.float32))
    beta = np.ascontiguousarray(np.asarray(gn_beta, np.float32))

    in_maps = []
    for g in range(N_CORES):
        idx = groups[g]
        xg = np.zeros((NPAD, C, T, H, W), np.float32)
        xg[:sizes[g]] = x[idx]
        xcm = xg.transpose(1, 0, 2, 3, 4).reshape(CC, 128, NPAD, T, H, W)
        xpp = np.zeros((CC, 128, NPAD, T, 10, 10), np.float32)
        xpp[..., 1:9, 1:9] = xcm
        xpp = xpp.reshape(CC, 128, NB, 8, T, 100)
        xp = np.ascontiguousarray(
            xpp.transpose(0, 1, 2, 4, 5, 3)).astype(ml_dtypes.bfloat16)
        # mask_big [120,120]: block-diagonal (3 locs), cols >= size -> -1e30
        mb = np.full((120, 120), -1e30, np.float32)
        for b in range(3):
            mb[b * NPAD:(b + 1) * NPAD, b * NPAD:b * NPAD + sizes[g]] = 0.0
        m = {"xp": xp, "maskb": mb, "gamma": gamma, "beta": beta}
        m.update(w_arrs)
        in_maps.append(m)

    if "nc" not in _BUILD_CACHE:
        _BUILD_CACHE["nc"] = _build()
    nc = _BUILD_CACHE["nc"]

    trace = bool(int(os.environ.get("ACAR_TRACE", "0")))
    if trace:
        _install_ntff_hook()
    res = run_bass_kernel_spmd(nc, in_maps, core_ids=list(range(N_CORES)),
                               trace=trace)
    LAST_EXEC_NS = res.exec_time_ns

    out = np.empty((N, C, T, H, W), np.float32)
    for g in range(N_CORES):
        og = res.results[g]["out"]  # [CC, 128, T, NB, (hw, n8)]
        og = og.reshape(C, T, NB, HW, 8)
        og = og.transpose(2, 4, 0, 1, 3).reshape(NPAD, C, T, H, W)
        out[groups[g]] = og[:sizes[g]]
    return out
